# revision 39
# baseline (speedup 1.0000x reference)
"""Trainium2 kernel for nn_AttLearner (retrieval_knn):
h = relu(x*w0)*w1; hn = h/||h||; adj = hn@hn.T; keep top-31 per row; relu.

Row-shard the [10000,10000] similarity matrix across 8 cores. Each core
holds the full (column-rotated) embedding set, computes a [1250,10000]
block, finds the per-row rank-31 threshold via hierarchical max8
(per-500-col top8 -> 160 candidates -> 4 rounds max8/match_replace), and
applies the mask with a fused (adj>=t)*adj scalar_tensor_tensor. Column
rotation per core makes the SPMD graph identical: every core's own rows
are always columns 0..1249.

Precision: top-k masks amplify matmul noise into full-size element
swaps. Measured on HW: bf16 1e-1 rel, fp16 3.7e-2, f32r (fp22) 3.1e-2
(its multiply path truncates products) -- all over the 2e-2 gate. bf16
matmuls with exact-bf16 inputs are bit-exact, so hn is split into two
bf16 limbs (B1 + R, ~18 mantissa bits) and
adj = B1@B1' + B1@R' + R@B1' (R@R' ~2^-18 is a smooth bias, dropped).
Measured rel err 6e-3.

Schedule: engine streams execute in program order, so the preamble
(normalize+split, elementwise-bound) is emitted interleaved with tile
0's matmul groups to keep the PE busy; the mask+store runs at quarter
granularity so the next tile's evictions unblock early.
"""

import numpy as np

N = 10000
D = 512
NCORES = 8
NB = N // NCORES          # 1250 rows per core
P = 125                   # rows per tile
RT = NB // P              # 10 row tiles
CH = 500                  # matmul column chunk (PSUM bank)
NCH = N // CH             # 20 chunks
GRP = 4                   # chunks per PSUM group (shared-lhsT runs)
NG = NCH // GRP           # 5 groups
CHP = 250                 # preamble column chunk
NCHP = N // CHP           # 40 preamble chunks
QM = N // 4               # mask/store quarter
KT = D // 128             # 4 contraction tiles
NEG = -1e30

_CACHE = {}


def _build_nc():
    import concourse.bacc as bacc
    import concourse.mybir as mybir
    from concourse.tile import TileContext

    dt = mybir.dt
    F32, BF16 = dt.float32, dt.bfloat16
    A = mybir.AluOpType
    AF = mybir.ActivationFunctionType

    # Bacc (not Bass): its compile() pass legalizes multi-sem waits via event
    # semaphores, which walrus's low per-instruction wait limits require.
    nc = bacc.Bacc("TRN2", target_bir_lowering=False, debug=False)
    # x.T pre-reshaped [KT, 128, N]: one 3D-AP DMA per preamble chunk
    xt_ext = nc.declare_dram_parameter("xt", [KT, 128, N], F32, isOutput=False)
    wf_ext = nc.declare_dram_parameter("wf", [128, KT], F32, isOutput=False)
    out_ext = nc.declare_dram_parameter("out", [NB, N], BF16, isOutput=True)

    with TileContext(nc) as tc:
        with (
            tc.tile_pool(name="sb", bufs=1) as sb,
            tc.tile_pool(name="bigp", bufs=1) as bigp,
            tc.tile_pool(name="hhp", bufs=2) as hhp,
            tc.tile_pool(name="rbp", bufs=1) as rbp,
            tc.tile_pool(name="smallp", bufs=2) as smallp,
            tc.tile_pool(name="candp", bufs=1) as candp,
            tc.tile_pool(name="ps_mm", bufs=7, space="PSUM") as ps_mm,
            tc.tile_pool(name="ps_pre", bufs=1, space="PSUM") as ps_pre,
        ):
            # persistent tensors
            B1 = sb.tile([128, KT, N], BF16, tag="B1")       # 80,000 B/part
            R = sb.tile([128, KT, N], BF16, tag="R")         # 80,000 B/part
            wsb = sb.tile([128, KT], F32, tag="wsb")
            ones_c = sb.tile([128, 1], F32, tag="ones_c")    # ss matmul lhsT
            ones_r = sb.tile([1, 128], F32, tag="ones_r")    # bcast matmul lhsT

            nc.sync.dma_start(out=wsb[:, :], in_=wf_ext[:, :])
            nc.vector.memset(ones_c[:, :], 1.0)
            nc.vector.memset(ones_r[:, :], 1.0)

            def preamble_chunk(j):
                """Produce B1/R limbs for columns [j*CHP, (j+1)*CHP)."""
                c0 = j * CHP
                hh = hhp.tile([128, KT, CHP], F32, tag="hh", name=f"hh_{j}")
                xs = xt_ext[:, :, c0:c0 + CHP].rearrange("k p c -> p k c")
                nc.sync.dma_start(out=hh[:, :, :], in_=xs)
                # h = max(x*weff, 0) in place, square in place
                for kt in range(KT):
                    nc.vector.tensor_scalar(
                        hh[:, kt, :], hh[:, kt, :], wsb[:, kt:kt + 1], 0.0,
                        op0=A.mult, op1=A.max,
                    )
                nc.scalar.activation(hh[:, :, :], hh[:, :, :], AF.Square)
                # ss[n] = sum_d h^2 via ones.T @ h^2, TRUE f32 matmul (exact)
                pb = ps_pre.tile([128, CHP], F32, tag="pre", name=f"pb_{j}")
                for kt in range(KT):
                    nc.tensor.matmul(pb[0:1, :], ones_c[:, :], hh[:, kt, :],
                                     start=(kt == 0), stop=(kt == KT - 1))
                # rnorm = 1/sqrt(ss): ACT sqrt (PSUM src) then DVE reciprocal
                srt = smallp.tile([1, CHP], F32, tag="srt", name=f"srt_{j}")
                nc.scalar.activation(srt[:, :], pb[0:1, :], AF.Sqrt)
                nc.vector.reciprocal(srt[:, :], srt[:, :])
                # broadcast to 128 partitions via K=1 TRUE-f32 matmul, reusing
                # the same PSUM bank (the ss row was consumed by the sqrt)
                nc.tensor.matmul(pb[:, :], ones_r[:, :], srt[:, :],
                                 start=True, stop=True)
                rb = rbp.tile([128, CHP], F32, tag="rb", name=f"rb_{j}")
                nc.scalar.copy(rb[:, :], pb[:, :])
                # reload x, p = (x*weff)*rb in place, then split:
                # B1 = bf16(relu(p)); R = bf16(max(p,0) - B1)
                nc.sync.dma_start(out=hh[:, :, :], in_=xs)
                for kt in range(KT):
                    nc.vector.scalar_tensor_tensor(
                        hh[:, kt, :], hh[:, kt, :], wsb[:, kt:kt + 1], rb[:, :],
                        op0=A.mult, op1=A.mult,
                    )
                b13 = B1[:, :, c0:c0 + CHP]
                nc.scalar.activation(b13, hh[:, :, :], AF.Relu)
                nc.vector.scalar_tensor_tensor(
                    R[:, :, c0:c0 + CHP], hh[:, :, :], 0.0, b13,
                    op0=A.max, op1=A.subtract,
                )

            def mm_group(t, adj, cand, j0, ngrp):
                """Matmuls+evict+L1 for row tile t, chunks [j0, j0+ngrp)."""
                r0 = t * P
                pms = [ps_mm.tile([128, CH], F32, tag="mm", name=f"pm_{t}_{j0}_{k}")
                       for k in range(ngrp)]
                # stationary-operand-major: consecutive matmuls share each
                # lhsT (PE reorder window hides the reloads)
                for term in range(3 * KT):
                    kt = term % KT
                    lhs_t = B1 if term < 2 * KT else R
                    rhs_t = R if KT <= term < 2 * KT else B1
                    lhs = lhs_t[:, kt, r0:r0 + P]
                    for jj in range(ngrp):
                        c0 = (j0 + jj) * CH
                        nc.tensor.matmul(
                            pms[jj][:P, :], lhs, rhs_t[:, kt, c0:c0 + CH],
                            start=(term == 0), stop=(term == 3 * KT - 1),
                        )
                for jj in range(ngrp):
                    j = j0 + jj
                    c0 = j * CH
                    nc.scalar.copy(adj[:P, c0:c0 + CH], pms[jj][:P, :])
                    nc.vector.max(out=cand[:P, 8 * j:8 * j + 8],
                                  in_=adj[:P, c0:c0 + CH])

            def select_mask_store(t, adj, cand):
                """Rank-31 threshold, mask in place, store (per quarter)."""
                r0 = t * P
                vals = None
                for r in range(4):
                    vals = smallp.tile([128, 8], F32, tag="vals",
                                       name=f"vals_{t}_{r}")
                    nc.vector.max(out=vals[:P, :], in_=cand[:P, :])
                    if r < 3:
                        nc.vector.match_replace(
                            out=cand[:P, :], in_to_replace=vals[:P, :],
                            in_values=cand[:P, :], imm_value=NEG,
                        )
                tcol = vals[:P, 6:7]  # rank 24+7 = 31
                # mask into bf16 staging pieces (reusing the preamble's hh
                # slots): the adj buffer is then released by the fast STT
                # reads, not by store DMAs, so the next tile's evictions
                # unblock early; bf16 also halves store traffic.
                PW = 2000
                for q in range(N // PW):
                    c = q * PW
                    ob = hhp.tile([128, PW], BF16, tag="hh", name=f"ob_{t}_{q}")
                    nc.vector.scalar_tensor_tensor(
                        ob[:P, :], adj[:P, c:c + PW], tcol, adj[:P, c:c + PW],
                        op0=A.is_ge, op1=A.mult,
                    )
                    nc.sync.dma_start(out=out_ext[r0:r0 + P, c:c + PW],
                                      in_=ob[:P, :])

            # tile 0 interleaved with the preamble so the PE has elementwise
            # shadow work while B1/R are being produced
            G0 = 4                      # tile-0 group size
            PPG = NCHP // (NCH // G0)   # preamble chunks per tile-0 group
            adj = bigp.tile([128, N], F32, tag="big", name="adj_0")
            cand = candp.tile([128, 8 * NCH], F32, tag="cand", name="cand_0")
            for g in range(NCH // G0):
                for jp in range(g * PPG, (g + 1) * PPG):
                    preamble_chunk(jp)
                mm_group(0, adj, cand, g * G0, G0)
            select_mask_store(0, adj, cand)

            for t in range(1, RT):
                adj = bigp.tile([128, N], F32, tag="big", name=f"adj_{t}")
                cand = candp.tile([128, 8 * NCH], F32, tag="cand", name=f"cand_{t}")
                for g in range(NG):
                    mm_group(t, adj, cand, g * GRP, GRP)
                select_mask_store(t, adj, cand)
    nc.finalize()  # Bacc: runs sync legalization + register allocation
    return nc


def _run(inputs, trace=False, trace_kwargs=None):
    from concourse.bass_utils import run_bass_kernel_spmd

    if "nc" not in _CACHE:
        _CACHE["nc"] = _build_nc()
    nc = _CACHE["nc"]

    x = np.asarray(inputs["x"], dtype=np.float32)
    w = np.asarray(inputs["w"], dtype=np.float32)
    weff = (w[0] * w[1]).astype(np.float32)
    wf = np.ascontiguousarray(weff.reshape(KT, 128).T)  # [128, KT]
    xT = np.ascontiguousarray(x.T)                      # [512, 10000]

    in_maps = []
    for i in range(NCORES):
        xr = np.ascontiguousarray(np.roll(xT, -i * NB, axis=1)).reshape(KT, 128, N)
        in_maps.append({"xt": xr, "wf": wf})

    kw = {}
    if trace:
        kw = dict(trace=True, trace_kwargs=trace_kwargs or {})
    res = run_bass_kernel_spmd(nc, in_maps, core_ids=list(range(NCORES)), **kw)

    full = np.empty((N, N), dtype=np.float32)
    for i in range(NCORES):
        blk = np.asarray(res.results[i]["out"]).astype(np.float32)
        full[i * NB:(i + 1) * NB, :] = np.roll(blk, i * NB, axis=1)
    return full, res


def kernel(**inputs) -> np.ndarray:
    out, _ = _run(inputs, trace=False)
    return out
